# revision 5
# baseline (speedup 1.0000x reference)
"""Distributed embedding-lookup kernel (doc2vec PV-DM forward) for 8 trn2 cores.

v4: transposed gathers + PE reduction.

Per-core compact bf16 sub-tables (unique rows only, <32768 so int16 indices
work with the custom dma_gather ucode):
  dcsub [18432,128] = [unique doc rows | unique ctx word rows]
  ssub  [20480,128] = unique sampled output columns (outputs pre-transposed)

Per group of M=4 tiles (512 batch rows = "tp" in 0..511):
  GAt [128d, 4608] <- dma_gather(transpose=True): cols [doc tp | ctx c*512+tp]
  GBt [128d, 5120] <- same: cols s*512+tp
  DVE (all bf16 2x-rate ops): ctx tree-sum -> inpT = cs/8 + docT,
      prodT[d, s, tp] = GBt * inpT (broadcast over s)
  PE : red[s][1, tp] = ones[128,1]^T @ prodT[:, s*512:(s+1)*512]
       (the d-reduction; bf16 in, f32 PSUM accumulate)
  ACT: drains each PSUM bank into redT[1, 5120] (strided: storage tp*10+s)
  one HWDGE write of redT -> res rows (row-major (tp, s) matches exactly)

This keeps the only 1x-rate DVE op (tensor_reduce) off the DVE entirely;
DVE work per group is ~4.6k cycles, hidden under the gather transfers
(measured ~54us/core for the 38912 random 256B row reads).
"""

import sys

if "/opt/trn_rl_repo" not in sys.path:
    sys.path.insert(0, "/opt/trn_rl_repo")

import numpy as np

N_CORES = 8
B, C, S = 16384, 8, 10
D = 128
P = 128
N_DOCS, N_WORDS = 200000, 100000
BS = B // N_CORES   # 2048 batch rows per core
T = BS // P         # 16 tiles of 128 rows per core
M = 4               # tiles per group
G_CNT = T // M      # 4 groups
TP = M * P          # 512 batch rows per group
DC_CAP = BS
CW_CAP = BS * C
DCSUB_ROWS = DC_CAP + CW_CAP   # 18432
SSUB_ROWS = BS * S             # 20480
NI_A = TP * (1 + C)            # 4608
NI_B = TP * S                  # 5120

_COMPILED = {}
LAST_RESULT = None


def build_program(reps=1):
    import concourse.bass as bass
    import concourse.tile as tile
    from concourse import bacc, mybir
    from contextlib import ExitStack

    f32 = mybir.dt.float32
    bf16 = mybir.dt.bfloat16
    i16 = mybir.dt.int16
    mult = mybir.AluOpType.mult
    add = mybir.AluOpType.add

    nc = bacc.Bacc(
        "TRN2",
        target_bir_lowering=False,
        debug=False,
        enable_asserts=False,
        num_devices=N_CORES,
    )

    dcsub_d = nc.dram_tensor("dcsub", [DCSUB_ROWS, D], bf16, kind="ExternalInput").ap()
    ssub_d = nc.dram_tensor("ssub", [SSUB_ROWS, D], bf16, kind="ExternalInput").ap()
    idxa_d = nc.dram_tensor("idxa", [P, G_CNT * (NI_A // 16)], i16, kind="ExternalInput").ap()
    idxb_d = nc.dram_tensor("idxb", [P, G_CNT * (NI_B // 16)], i16, kind="ExternalInput").ap()
    res_d = nc.dram_tensor("res", [BS, S], f32, kind="ExternalOutput").ap()

    CA = NI_A // 16  # 288
    CB = NI_B // 16  # 320

    with tile.TileContext(nc) as tc, ExitStack() as ctx:
        idxp = ctx.enter_context(tc.tile_pool(name="idxp", bufs=1))
        gat = ctx.enter_context(tc.tile_pool(name="gat", bufs=3))
        cmp_p = ctx.enter_context(tc.tile_pool(name="cmp", bufs=2))
        outp = ctx.enter_context(tc.tile_pool(name="outp", bufs=2))
        psum_p = ctx.enter_context(tc.tile_pool(name="psum", bufs=4, space="PSUM"))

        idxa = idxp.tile([P, G_CNT * CA], i16, name="idxa")
        nc.sync.dma_start(out=idxa[:], in_=idxa_d)
        idxb = idxp.tile([P, G_CNT * CB], i16, name="idxb")
        nc.sync.dma_start(out=idxb[:], in_=idxb_d)
        ones_t = idxp.tile([P, 1], bf16, name="ones")
        nc.vector.memset(ones_t[:], 1.0)

        def body():
            for g in range(G_CNT):
                GAt = gat.tile([P, NI_A], bf16, tag="GAt", name="GAt")
                nc.gpsimd.dma_gather(
                    out_ap=GAt[:].rearrange("p (o n) -> p o n", o=1, n=NI_A),
                    in_ap=dcsub_d,
                    idxs_ap=idxa[:, g * CA : (g + 1) * CA],
                    num_idxs=NI_A,
                    num_idxs_reg=NI_A,
                    elem_size=D,
                    transpose=True,
                    single_packet=False,
                )
                GBt = gat.tile([P, NI_B], bf16, tag="GBt", name="GBt")
                nc.gpsimd.dma_gather(
                    out_ap=GBt[:].rearrange("p (o n) -> p o n", o=1, n=NI_B),
                    in_ap=ssub_d,
                    idxs_ap=idxb[:, g * CB : (g + 1) * CB],
                    num_idxs=NI_B,
                    num_idxs_reg=NI_B,
                    elem_size=D,
                    transpose=True,
                    single_packet=False,
                )

                docT = GAt[:, 0:TP]
                ctxb = GAt[:, TP : (1 + C) * TP]

                t1 = cmp_p.tile([P, 4 * TP], bf16, tag="t1", name="t1")
                nc.vector.tensor_add(
                    out=t1[:], in0=ctxb[:, 0 : 4 * TP], in1=ctxb[:, 4 * TP : 8 * TP]
                )
                t2 = cmp_p.tile([P, 2 * TP], bf16, tag="t2", name="t2")
                nc.vector.tensor_add(
                    out=t2[:], in0=t1[:, 0 : 2 * TP], in1=t1[:, 2 * TP : 4 * TP]
                )
                cs = cmp_p.tile([P, TP], bf16, tag="cs", name="cs")
                nc.vector.tensor_add(out=cs[:], in0=t2[:, 0:TP], in1=t2[:, TP : 2 * TP])

                inpT = cmp_p.tile([P, TP], bf16, tag="inpT", name="inpT")
                nc.vector.scalar_tensor_tensor(
                    out=inpT[:],
                    in0=cs[:],
                    scalar=1.0 / C,
                    in1=docT,
                    op0=mult,
                    op1=add,
                )

                prodT = cmp_p.tile([P, S * TP], bf16, tag="prodT", name="prodT")
                smp3 = GBt[:].rearrange("p (s tp) -> p s tp", s=S, tp=TP)
                inp3 = inpT[:].unsqueeze(1).to_broadcast([P, S, TP])
                prod3 = prodT[:].rearrange("p (s tp) -> p s tp", s=S, tp=TP)
                nc.vector.tensor_tensor(out=prod3, in0=smp3, in1=inp3, op=mult)

                redT = outp.tile([1, S * TP], f32, tag="redT", name="redT")
                # 3D view [1, tp, s] of the tp*10+s storage for strided drains
                redT3 = redT[:].rearrange("p (tp s) -> p tp s", tp=TP, s=S)
                for h in range(S // 2):
                    ps = psum_p.tile([1, 2 * TP], f32, tag="ps", name="ps", space="PSUM")
                    for j in range(2):
                        s = 2 * h + j
                        nc.tensor.matmul(
                            out=ps[:, j * TP : (j + 1) * TP],
                            lhsT=ones_t[:],
                            rhs=prodT[:, s * TP : (s + 1) * TP],
                            start=True,
                            stop=True,
                        )
                    nc.scalar.copy(
                        out=redT3[:, :, 2 * h : 2 * h + 2],
                        in_=ps[:].rearrange("p (s tp) -> p tp s", s=2, tp=TP),
                    )

                dst = res_d[g * TP : (g + 1) * TP, :].rearrange(
                    "(o r) s -> o (r s)", o=1
                )
                nc.sync.dma_start(out=dst, in_=redT[:])

        if reps == 1:
            body()
        else:
            with tc.For_i(0, reps) as _i:
                body()

    nc.compile()
    return nc


def _get_program():
    if "nc" not in _COMPILED:
        _COMPILED["nc"] = build_program(1)
    return _COMPILED["nc"]


def _wrap16(pos_list):
    """[N] -> [128, N/16] int16: (ch, col) = pos[col*16+ch], replicated 8x
    (one copy per 16-partition group for the 8 Q7 descriptor-gen cores)."""
    w = np.asarray(pos_list, np.int16).reshape(-1, 16).T
    return np.tile(w, (8, 1))


def make_in_maps(doc_ids, context_ids, sample_ids, paragraph_matrix, word_matrix, outputs):
    import ml_dtypes

    bf = ml_dtypes.bfloat16
    par = np.asarray(paragraph_matrix, dtype=np.float32).astype(bf)
    wrd = np.asarray(word_matrix, dtype=np.float32).astype(bf)
    outT = np.ascontiguousarray(np.asarray(outputs, dtype=np.float32).T).astype(bf)
    doc_ids = np.asarray(doc_ids)
    context_ids = np.asarray(context_ids)
    sample_ids = np.asarray(sample_ids)

    in_maps = []
    for k in range(N_CORES):
        sl = slice(k * BS, (k + 1) * BS)
        du, dinv = np.unique(doc_ids[sl], return_inverse=True)
        cu, cinv = np.unique(context_ids[sl].ravel(), return_inverse=True)
        su, sinv = np.unique(sample_ids[sl].ravel(), return_inverse=True)
        assert len(du) <= DC_CAP and len(cu) <= CW_CAP and len(su) <= SSUB_ROWS

        dcsub = np.zeros((DCSUB_ROWS, D), bf)
        dcsub[: len(du)] = par[du]
        dcsub[DC_CAP : DC_CAP + len(cu)] = wrd[cu]
        ssub = np.zeros((SSUB_ROWS, D), bf)
        ssub[: len(su)] = outT[su]

        d = dinv.reshape(G_CNT, TP)                       # [g, tp]
        c = (cinv.reshape(G_CNT, TP, C) + DC_CAP)         # [g, tp, c]
        s = sinv.reshape(G_CNT, TP, S)                    # [g, tp, s]

        # gather A positions: [doc tp | ctx c*TP+tp]  (c-major)
        posA = np.concatenate(
            [d[:, None, :], c.transpose(0, 2, 1)], axis=1
        )                                                 # [g, 1+C, tp]
        # gather B positions: s*TP+tp  (s-major)
        posB = s.transpose(0, 2, 1)                       # [g, S, tp]

        idxa = np.concatenate([_wrap16(posA[g].ravel()) for g in range(G_CNT)], axis=1)
        idxb = np.concatenate([_wrap16(posB[g].ravel()) for g in range(G_CNT)], axis=1)
        in_maps.append(
            {
                "dcsub": dcsub,
                "ssub": ssub,
                "idxa": np.ascontiguousarray(idxa),
                "idxb": np.ascontiguousarray(idxb),
            }
        )
    return in_maps


def unshard_result(res_list):
    return np.concatenate(res_list, axis=0).astype(np.float32)


def kernel(
    doc_ids,
    context_ids,
    sample_ids,
    paragraph_matrix,
    word_matrix,
    outputs,
) -> np.ndarray:
    global LAST_RESULT
    from concourse.bass_utils import run_bass_kernel_spmd

    nc = _get_program()
    in_maps = make_in_maps(
        doc_ids, context_ids, sample_ids, paragraph_matrix, word_matrix, outputs
    )
    LAST_RESULT = run_bass_kernel_spmd(nc, in_maps, list(range(N_CORES)))
    return unshard_result(
        [LAST_RESULT.results[k]["res"] for k in range(N_CORES)]
    )


# revision 6
# speedup vs baseline: 4.1498x; 4.1498x over previous
"""Distributed embedding-lookup kernel (doc2vec PV-DM forward) for 8 trn2 cores.

v4: transposed gathers + PE reduction.

Per-core compact bf16 sub-tables (unique rows only, <32768 so int16 indices
work with the custom dma_gather ucode):
  dcsub [18432,128] = [unique doc rows | unique ctx word rows]
  ssub  [20480,128] = unique sampled output columns (outputs pre-transposed)

Per group of M=4 tiles (512 batch rows = "tp" in 0..511):
  GAt [128d, 4608] <- dma_gather(transpose=True): cols [doc tp | ctx c*512+tp]
  GBt [128d, 5120] <- same: cols s*512+tp
  DVE (all bf16 2x-rate ops): ctx tree-sum -> inpT = cs/8 + docT,
      prodT[d, s, tp] = GBt * inpT (broadcast over s)
  PE : red[s][1, tp] = ones[128,1]^T @ prodT[:, s*512:(s+1)*512]
       (the d-reduction; bf16 in, f32 PSUM accumulate)
  ACT: drains each PSUM bank into redT[1, 5120] (strided: storage tp*10+s)
  one HWDGE write of redT -> res rows (row-major (tp, s) matches exactly)

This keeps the only 1x-rate DVE op (tensor_reduce) off the DVE entirely;
DVE work per group is ~4.6k cycles, hidden under the gather transfers
(measured ~54us/core for the 38912 random 256B row reads).
"""

import sys

if "/opt/trn_rl_repo" not in sys.path:
    sys.path.insert(0, "/opt/trn_rl_repo")

import numpy as np

N_CORES = 8
B, C, S = 16384, 8, 10
D = 128
P = 128
N_DOCS, N_WORDS = 200000, 100000
BS = B // N_CORES   # 2048 batch rows per core
T = BS // P         # 16 tiles of 128 rows per core
M = 4               # tiles per group
G_CNT = T // M      # 4 groups
TP = M * P          # 512 batch rows per group
DC_CAP = BS
CW_CAP = BS * C
DCSUB_ROWS = DC_CAP + CW_CAP   # 18432
SSUB_ROWS = BS * S             # 20480
NI_A = TP * (1 + C)            # 4608
NI_B = TP * S                  # 5120

_COMPILED = {}
LAST_RESULT = None


def build_program(reps=1):
    import concourse.bass as bass
    import concourse.tile as tile
    from concourse import bacc, mybir
    from contextlib import ExitStack

    f32 = mybir.dt.float32
    bf16 = mybir.dt.bfloat16
    i16 = mybir.dt.int16
    mult = mybir.AluOpType.mult
    add = mybir.AluOpType.add

    nc = bacc.Bacc(
        "TRN2",
        target_bir_lowering=False,
        debug=False,
        enable_asserts=False,
        num_devices=N_CORES,
    )

    dcsub_d = nc.dram_tensor("dcsub", [DCSUB_ROWS, D], bf16, kind="ExternalInput").ap()
    ssub_d = nc.dram_tensor("ssub", [SSUB_ROWS, D], bf16, kind="ExternalInput").ap()
    idxa_d = nc.dram_tensor("idxa", [P, G_CNT * (NI_A // 16)], i16, kind="ExternalInput").ap()
    idxb_d = nc.dram_tensor("idxb", [P, G_CNT * (NI_B // 16)], i16, kind="ExternalInput").ap()
    res_d = nc.dram_tensor("res", [BS, S], f32, kind="ExternalOutput").ap()

    CA = NI_A // 16  # 288
    CB = NI_B // 16  # 320

    with tile.TileContext(nc) as tc, ExitStack() as ctx:
        idxp = ctx.enter_context(tc.tile_pool(name="idxp", bufs=1))
        gat = ctx.enter_context(tc.tile_pool(name="gat", bufs=3))
        cmp_p = ctx.enter_context(tc.tile_pool(name="cmp", bufs=2))
        outp = ctx.enter_context(tc.tile_pool(name="outp", bufs=2))
        psum_p = ctx.enter_context(tc.tile_pool(name="psum", bufs=4, space="PSUM"))

        idxa = idxp.tile([P, G_CNT * CA], i16, name="idxa")
        nc.sync.dma_start(out=idxa[:], in_=idxa_d)
        idxb = idxp.tile([P, G_CNT * CB], i16, name="idxb")
        nc.sync.dma_start(out=idxb[:], in_=idxb_d)
        ones_t = idxp.tile([P, 1], bf16, name="ones")
        nc.vector.memset(ones_t[:], 1.0)

        def body():
            for g in range(G_CNT):
                GAt = gat.tile([P, NI_A], bf16, tag="GAt", name="GAt")
                nc.gpsimd.dma_gather(
                    out_ap=GAt[:].rearrange("p (o n) -> p o n", o=1, n=NI_A),
                    in_ap=dcsub_d,
                    idxs_ap=idxa[:, g * CA : (g + 1) * CA],
                    num_idxs=NI_A,
                    num_idxs_reg=NI_A,
                    elem_size=D,
                    transpose=True,
                    single_packet=False,
                )
                GBt = gat.tile([P, NI_B], bf16, tag="GBt", name="GBt")
                nc.gpsimd.dma_gather(
                    out_ap=GBt[:].rearrange("p (o n) -> p o n", o=1, n=NI_B),
                    in_ap=ssub_d,
                    idxs_ap=idxb[:, g * CB : (g + 1) * CB],
                    num_idxs=NI_B,
                    num_idxs_reg=NI_B,
                    elem_size=D,
                    transpose=True,
                    single_packet=False,
                )

                docT = GAt[:, 0:TP]
                ctxb = GAt[:, TP : (1 + C) * TP]

                t1 = cmp_p.tile([P, 4 * TP], bf16, tag="t1", name="t1")
                nc.vector.tensor_add(
                    out=t1[:], in0=ctxb[:, 0 : 4 * TP], in1=ctxb[:, 4 * TP : 8 * TP]
                )
                t2 = cmp_p.tile([P, 2 * TP], bf16, tag="t2", name="t2")
                nc.vector.tensor_add(
                    out=t2[:], in0=t1[:, 0 : 2 * TP], in1=t1[:, 2 * TP : 4 * TP]
                )
                cs = cmp_p.tile([P, TP], bf16, tag="cs", name="cs")
                nc.vector.tensor_add(out=cs[:], in0=t2[:, 0:TP], in1=t2[:, TP : 2 * TP])

                inpT = cmp_p.tile([P, TP], bf16, tag="inpT", name="inpT")
                nc.vector.scalar_tensor_tensor(
                    out=inpT[:],
                    in0=cs[:],
                    scalar=1.0 / C,
                    in1=docT,
                    op0=mult,
                    op1=add,
                )

                prodT = cmp_p.tile([P, S * TP], bf16, tag="prodT", name="prodT")
                smp3 = GBt[:].rearrange("p (tp s) -> p tp s", tp=TP, s=S)
                inp3 = inpT[:].unsqueeze(2).to_broadcast([P, TP, S])
                prod3 = prodT[:].rearrange("p (tp s) -> p tp s", tp=TP, s=S)
                nc.vector.tensor_tensor(out=prod3, in0=smp3, in1=inp3, op=mult)

                redT = outp.tile([1, S * TP], f32, tag="redT", name="redT")
                for h in range(S // 2):
                    ps = psum_p.tile([1, 2 * TP], f32, tag="ps", name="ps", space="PSUM")
                    for j in range(2):
                        k = 2 * h + j
                        nc.tensor.matmul(
                            out=ps[:, j * TP : (j + 1) * TP],
                            lhsT=ones_t[:],
                            rhs=prodT[:, k * TP : (k + 1) * TP],
                            start=True,
                            stop=True,
                        )
                    nc.scalar.copy(
                        out=redT[0:1, h * 2 * TP : (h + 1) * 2 * TP], in_=ps[:]
                    )

                dst = res_d[g * TP : (g + 1) * TP, :].rearrange(
                    "(o r) s -> o (r s)", o=1
                )
                nc.sync.dma_start(out=dst, in_=redT[:])

        if reps == 1:
            body()
        else:
            with tc.For_i(0, reps) as _i:
                body()

    nc.compile()
    return nc


def _get_program():
    if "nc" not in _COMPILED:
        _COMPILED["nc"] = build_program(1)
    return _COMPILED["nc"]


def _wrap16(pos_list):
    """[N] -> [128, N/16] int16: (ch, col) = pos[col*16+ch], replicated 8x
    (one copy per 16-partition group for the 8 Q7 descriptor-gen cores)."""
    w = np.asarray(pos_list, np.int16).reshape(-1, 16).T
    return np.tile(w, (8, 1))


def make_in_maps(doc_ids, context_ids, sample_ids, paragraph_matrix, word_matrix, outputs):
    import ml_dtypes

    bf = ml_dtypes.bfloat16
    par = np.asarray(paragraph_matrix, dtype=np.float32).astype(bf)
    wrd = np.asarray(word_matrix, dtype=np.float32).astype(bf)
    outT = np.ascontiguousarray(np.asarray(outputs, dtype=np.float32).T).astype(bf)
    doc_ids = np.asarray(doc_ids)
    context_ids = np.asarray(context_ids)
    sample_ids = np.asarray(sample_ids)

    in_maps = []
    for k in range(N_CORES):
        sl = slice(k * BS, (k + 1) * BS)
        du, dinv = np.unique(doc_ids[sl], return_inverse=True)
        cu, cinv = np.unique(context_ids[sl].ravel(), return_inverse=True)
        su, sinv = np.unique(sample_ids[sl].ravel(), return_inverse=True)
        assert len(du) <= DC_CAP and len(cu) <= CW_CAP and len(su) <= SSUB_ROWS

        dcsub = np.zeros((DCSUB_ROWS, D), bf)
        dcsub[: len(du)] = par[du]
        dcsub[DC_CAP : DC_CAP + len(cu)] = wrd[cu]
        ssub = np.zeros((SSUB_ROWS, D), bf)
        ssub[: len(su)] = outT[su]

        d = dinv.reshape(G_CNT, TP)                       # [g, tp]
        c = (cinv.reshape(G_CNT, TP, C) + DC_CAP)         # [g, tp, c]
        s = sinv.reshape(G_CNT, TP, S)                    # [g, tp, s]

        # gather A positions: [doc tp | ctx c*TP+tp]  (c-major)
        posA = np.concatenate(
            [d[:, None, :], c.transpose(0, 2, 1)], axis=1
        )                                                 # [g, 1+C, tp]
        # gather B positions: tp*S+s  (tp-major, matches res row-major)
        posB = s                                          # [g, tp, S]

        idxa = np.concatenate([_wrap16(posA[g].ravel()) for g in range(G_CNT)], axis=1)
        idxb = np.concatenate([_wrap16(posB[g].ravel()) for g in range(G_CNT)], axis=1)
        in_maps.append(
            {
                "dcsub": dcsub,
                "ssub": ssub,
                "idxa": np.ascontiguousarray(idxa),
                "idxb": np.ascontiguousarray(idxb),
            }
        )
    return in_maps


def unshard_result(res_list):
    return np.concatenate(res_list, axis=0).astype(np.float32)


def kernel(
    doc_ids,
    context_ids,
    sample_ids,
    paragraph_matrix,
    word_matrix,
    outputs,
) -> np.ndarray:
    global LAST_RESULT
    from concourse.bass_utils import run_bass_kernel_spmd

    nc = _get_program()
    in_maps = make_in_maps(
        doc_ids, context_ids, sample_ids, paragraph_matrix, word_matrix, outputs
    )
    LAST_RESULT = run_bass_kernel_spmd(nc, in_maps, list(range(N_CORES)))
    return unshard_result(
        [LAST_RESULT.results[k]["res"] for k in range(N_CORES)]
    )
